# revision 6
# baseline (speedup 1.0000x reference)
"""MoE (top-2 of 8 experts + shared expert) Trainium2 kernel, expert-parallel
across 8 NeuronCores.

Strategy (per sharding hint): the gate is evaluated on host as part of the
dispatch/sharding step (0.1% of total FLOPs); tokens are gathered per top-k
expert id and shipped to the core owning that expert's FFN weights. Each core
runs the two-matmul gelu FFN (float32r matmuls, fp32 PSUM accumulate) over its
capacity-padded token set, applies the combine weight per token, and also
computes the shared-expert projection for a 1/8 block of tokens
(data-parallel). Host scatters the per-expert outputs back and adds residual +
shared + routed contributions. Load-balance stats (f, p) are O(N*E) counts
computed on host.
"""
import numpy as np

import concourse.bass as bass
import concourse.mybir as mybir
import concourse.tile as tile
from concourse import bacc
from concourse.bass_utils import run_bass_kernel_spmd

P = 128
TT = 512            # token tile (matmul free dim)
N_CORES = 8

F32 = mybir.dt.float32
F32R = mybir.dt.float32r
AF = mybir.ActivationFunctionType
ALU = mybir.AluOpType

_PROG_CACHE = {}


def _build_program(C, H, cap, tb):
    """SPMD program for one core: expert FFN over `cap` gathered tokens plus
    shared-expert projection over a `tb`-token block.

    Layouts (all DRAM params pre-tiled on host, partition dim first):
      xT    [P, C/P, cap]   x^T for gathered tokens  (k-major for matmul 1)
      w1    [P, C/P, H]     W1[e]                    (lhsT tiles for matmul 1)
      b1t   [P, H/P]        b1[e], h = mo*P + p
      w2    [P, H/P, C]     W2[e]                    (rhs for matmul 2)
      b2b   [P, C]          b2[e] broadcast across partitions
      wslot [P, cap/P]      combine weight per token slot, t = mo*P + p
      xsT   [P, C/P, tb]    x^T for this core's shared-expert token block
      wsh   [P, C/P, C]     sum-over-heads shared weight Ws_eff
      bshb  [P, C]          bs_eff broadcast across partitions
    Outputs:
      y     [P, cap/P, C]   wslot * (gelu(x@W1+b1) @ W2 + b2), t = mo*P + p
      sh    [P, tb/P, C]    x@Ws_eff + bs_eff for the block, t = mo*P + p
    """
    KO1 = C // P          # k-subtiles for matmul 1 (contraction over C)
    MO1 = H // P          # m-subtiles for matmul 1 / k-subtiles for matmul 2
    MOY = cap // P        # token m-subtiles overall
    NT = cap // TT        # token tiles (expert FFN)
    NTS = tb // TT        # token tiles (shared expert)
    MT = TT // P          # token m-subtiles per token tile

    nc = bacc.Bacc("TRN2", target_bir_lowering=False, debug=False,
                   num_devices=N_CORES)
    xT = nc.declare_dram_parameter("xT", [P, KO1, cap], F32R, isOutput=False)
    w1 = nc.declare_dram_parameter("w1", [P, KO1, H], F32R, isOutput=False)
    b1t = nc.declare_dram_parameter("b1t", [P, MO1], F32, isOutput=False)
    w2 = nc.declare_dram_parameter("w2", [P, MO1, C], F32R, isOutput=False)
    b2b = nc.declare_dram_parameter("b2b", [P, C], F32, isOutput=False)
    wslot = nc.declare_dram_parameter("wslot", [P, MOY], F32, isOutput=False)
    xsT = nc.declare_dram_parameter("xsT", [P, KO1, tb], F32R, isOutput=False)
    wsh = nc.declare_dram_parameter("wsh", [P, KO1, C], F32R, isOutput=False)
    bshb = nc.declare_dram_parameter("bshb", [P, C], F32, isOutput=False)
    y = nc.declare_dram_parameter("y", [P, MOY, C], F32, isOutput=True)
    sh = nc.declare_dram_parameter("sh", [P, tb // P, C], F32, isOutput=True)

    with tile.TileContext(nc) as tc:
        with tc.tile_pool(name="const", bufs=1) as const, \
             tc.tile_pool(name="xp", bufs=3) as xp, \
             tc.tile_pool(name="xsp", bufs=2) as xsp, \
             tc.tile_pool(name="hp", bufs=2) as hp, \
             tc.tile_pool(name="yp", bufs=4) as yp, \
             tc.tile_pool(name="psA", bufs=3, space="PSUM") as psA, \
             tc.tile_pool(name="psB", bufs=2, space="PSUM") as psB:
            # Shared-expert weights first: the shared phase runs while the
            # (much larger) expert weights stream in.
            wsh_sb = const.tile([P, KO1, C], F32R)
            nc.sync.dma_start(out=wsh_sb[:], in_=wsh[:])
            bshb_sb = const.tile([P, C], F32)
            nc.sync.dma_start(out=bshb_sb[:], in_=bshb[:])
            w1_sb = const.tile([P, KO1, H], F32R)
            nc.sync.dma_start(out=w1_sb[:], in_=w1[:])
            w2_sb = const.tile([P, MO1, C], F32R)
            nc.sync.dma_start(out=w2_sb[:], in_=w2[:])
            b1_sb = const.tile([P, MO1], F32)
            nc.sync.dma_start(out=b1_sb[:], in_=b1t[:])
            b2b_sb = const.tile([P, C], F32)
            nc.sync.dma_start(out=b2b_sb[:], in_=b2b[:])
            ws_sb = const.tile([P, MOY], F32)
            nc.sync.dma_start(out=ws_sb[:], in_=wslot[:])

            # ---- shared expert: sh[t, :] = x_blk @ Ws_eff + bs_eff ----
            for j in range(NTS):
                xst = xsp.tile([P, KO1, TT], F32R)
                nc.sync.dma_start(out=xst[:], in_=xsT[:, :, j * TT:(j + 1) * TT])
                for mt in range(MT):
                    ps = psB.tile([P, C], F32)
                    for ko in range(KO1):
                        nc.tensor.matmul(
                            ps[:],
                            lhsT=xst[:, ko, mt * P:(mt + 1) * P],
                            rhs=wsh_sb[:, ko, :],
                            start=(ko == 0), stop=(ko == KO1 - 1),
                        )
                    st = yp.tile([P, C], F32, tag="evict")
                    nc.vector.tensor_tensor(st[:], ps[:], bshb_sb[:], ALU.add)
                    nc.sync.dma_start(out=sh[:, j * MT + mt, :], in_=st[:])

            # ---- expert FFN over gathered tokens ----
            for j in range(NT):
                xt = xp.tile([P, KO1, TT], F32R)
                nc.sync.dma_start(out=xt[:], in_=xT[:, :, j * TT:(j + 1) * TT])
                ht = hp.tile([P, MO1, TT], F32R)
                for mo in range(MO1):
                    ps = psA.tile([P, TT], F32)
                    for ko in range(KO1):
                        nc.tensor.matmul(
                            ps[:],
                            lhsT=w1_sb[:, ko, mo * P:(mo + 1) * P],
                            rhs=xt[:, ko, :],
                            start=(ko == 0), stop=(ko == KO1 - 1),
                        )
                    nc.scalar.activation(
                        ht[:, mo, :], ps[:], AF.Gelu_apprx_tanh,
                        bias=b1_sb[:, mo:mo + 1], scale=1.0,
                    )
                for mt in range(MT):
                    ps2 = psB.tile([P, C], F32)
                    for ko in range(MO1):
                        nc.tensor.matmul(
                            ps2[:],
                            lhsT=ht[:, ko, mt * P:(mt + 1) * P],
                            rhs=w2_sb[:, ko, :],
                            start=(ko == 0), stop=(ko == MO1 - 1),
                        )
                    g = j * MT + mt
                    yt = yp.tile([P, C], F32, tag="evict")
                    nc.vector.tensor_tensor(yt[:], ps2[:], b2b_sb[:], ALU.add)
                    nc.vector.tensor_scalar_mul(yt[:], yt[:], ws_sb[:, g:g + 1])
                    nc.sync.dma_start(out=y[:, g, :], in_=yt[:])
    nc.compile()
    return nc


def _get_program(C, H, cap, tb):
    key = (C, H, cap, tb)
    if key not in _PROG_CACHE:
        _PROG_CACHE[key] = _build_program(C, H, cap, tb)
    return _PROG_CACHE[key]


def _ktile(a, P=128):
    """[K, M] -> [P, K/P, M] with k = ko*P + p."""
    K, M = a.shape
    return np.ascontiguousarray(a.reshape(K // P, P, M).transpose(1, 0, 2))


def kernel(x, Wg, bg, Ws, bs, W1, b1, W2, b2):
    out, _ = _run(x, Wg, bg, Ws, bs, W1, b1, W2, b2, trace=False)
    return out


def profile_once(inputs):
    """Run once with NTFF tracing; returns HW exec time in ns (or None)."""
    _, t = _run(**inputs, trace=True)
    return t


def _run(x, Wg, bg, Ws, bs, W1, b1, W2, b2, trace=False):
    x = np.asarray(x, dtype=np.float32)
    Wg = np.asarray(Wg, dtype=np.float32)
    bg = np.asarray(bg, dtype=np.float32)
    Ws = np.asarray(Ws, dtype=np.float32)
    bs = np.asarray(bs, dtype=np.float32)
    W1 = np.asarray(W1, dtype=np.float32)
    b1 = np.asarray(b1, dtype=np.float32)
    W2 = np.asarray(W2, dtype=np.float32)
    b2 = np.asarray(b2, dtype=np.float32)

    B, T, C = x.shape
    E = Wg.shape[-1]
    H = W1.shape[-1]
    NS = Ws.shape[-1] // C
    KK = 2                      # top-k
    N = B * T
    TB = N // N_CORES           # shared-expert block per core
    assert E == N_CORES and C % P == 0 and H % P == 0 and TB % TT == 0

    x_flat = x.reshape(N, C)

    # ---- gate (host; part of the dispatch/sharding step) ----
    z = x_flat.astype(np.float64) @ Wg.astype(np.float64) + bg.astype(np.float64)
    s = (1.0 / (1.0 + np.exp(-z))).astype(np.float32)          # [N, E]
    idx = np.argsort(-s, axis=-1, kind='stable')[:, :KK]       # [N, K] top-k ids
    g_top = np.take_along_axis(s, idx, axis=-1)                # [N, K]
    wn = g_top / g_top.sum(axis=-1, keepdims=True)             # combine weights

    # ---- dispatch: group (token, k) pairs by expert ----
    pair_e = idx.reshape(-1)                                   # [N*K]
    order = np.argsort(pair_e, kind='stable')
    counts = np.bincount(pair_e, minlength=E)
    bounds = np.concatenate(([0], np.cumsum(counts)))
    max_cnt = int(counts.max())
    cap = max(4608, -(-max_cnt // TT) * TT)                    # capacity (mult of TT)

    nc = _get_program(C, H, cap, TB)

    Ws_eff = Ws.reshape(C, NS, C).sum(axis=1)
    bs_eff = bs.reshape(NS, C).sum(axis=0)
    wsh_t = _ktile(Ws_eff)
    bshb = np.ascontiguousarray(np.broadcast_to(bs_eff, (P, C)))
    wn_flat = wn.reshape(-1)

    in_maps = []
    sels = []
    for e in range(E):
        sel = order[bounds[e]:bounds[e + 1]]
        sels.append(sel)
        cnt = sel.shape[0]
        tok = sel // KK
        x_g = np.zeros((cap, C), dtype=np.float32)
        x_g[:cnt] = x_flat[tok]
        wv = np.zeros(cap, dtype=np.float32)
        wv[:cnt] = wn_flat[sel]
        x_blk = x_flat[e * TB:(e + 1) * TB]
        in_maps.append({
            "xT": _ktile(np.ascontiguousarray(x_g.T)),
            "w1": _ktile(W1[e]),
            "b1t": np.ascontiguousarray(b1[e].reshape(H // P, P).T),
            "w2": _ktile(W2[e]),
            "b2b": np.ascontiguousarray(np.broadcast_to(b2[e], (P, C))),
            "wslot": np.ascontiguousarray(wv.reshape(cap // P, P).T),
            "xsT": np.ascontiguousarray(
                x_blk.T.reshape(C // P, P, TB).transpose(1, 0, 2)),
            "wsh": wsh_t,
            "bshb": bshb,
        })

    global _LAST_IN_MAPS
    _LAST_IN_MAPS = in_maps
    r = run_bass_kernel_spmd(nc, in_maps, core_ids=list(range(N_CORES)),
                             trace=trace)
    results = r.results

    # ---- unshard + combine ----
    y_pairs = np.empty((N * KK, C), dtype=np.float32)
    sh_all = np.empty((N, C), dtype=np.float32)
    for e in range(E):
        sel = sels[e]
        y_tok = results[e]["y"].transpose(1, 0, 2).reshape(cap, C)
        y_pairs[sel] = y_tok[:sel.shape[0]]
        sh_all[e * TB:(e + 1) * TB] = \
            results[e]["sh"].transpose(1, 0, 2).reshape(TB, C)

    route = y_pairs.reshape(N, KK, C).sum(axis=1)
    res = (x_flat + sh_all + route).reshape(B, T, C)

    # ---- load-balance stats (host, counts over N*K pairs) ----
    s_norm = s / s.sum(axis=-1, keepdims=True)
    s_top = np.take_along_axis(s_norm, idx, axis=-1)           # [N, K]
    idx_b = idx.reshape(B, T * KK)
    s_top_b = s_top.reshape(B, T * KK).astype(np.float64)
    f = np.empty((B, E), dtype=np.float32)
    p = np.empty((B, E), dtype=np.float32)
    for b in range(B):
        cnt_be = np.bincount(idx_b[b], minlength=E)
        wsum_be = np.bincount(idx_b[b], weights=s_top_b[b], minlength=E)
        f[b] = (T * KK - cnt_be).astype(np.float32)
        p[b] = (s_top_b[b].sum() - wsum_be).astype(np.float32)
    return (res, f, p), r.exec_time_ns


# revision 9
# speedup vs baseline: 1.0999x; 1.0999x over previous
"""MoE (top-2 of 8 experts + shared expert) Trainium2 kernel, expert-parallel
across 8 NeuronCores.

Strategy (per sharding hint): the gate is evaluated on host as part of the
dispatch/sharding step (0.1% of total FLOPs); tokens are gathered per top-k
expert id and shipped to the core owning that expert's FFN weights. Each core
runs the two-matmul gelu FFN (float32r matmuls, fp32 PSUM accumulate) over its
capacity-padded token set, applies the combine weight per token, and also
computes the shared-expert projection for a 1/8 block of tokens
(data-parallel). Host scatters the per-expert outputs back and adds residual +
shared + routed contributions. Load-balance stats (f, p) are O(N*E) counts
computed on host.

Device layouts (all DRAM params pre-tiled on host, partition dim first):
  xT    [P, C/P, cap]   x^T for gathered tokens  (k-major for matmul 1)
  w1    [P, C/P, H]     W1[e]                    (lhsT tiles for matmul 1)
  b1t   [P, H/P]        b1[e], h = mo*P + p
  w2    [P, H/P, C]     W2[e]                    (rhs for matmul 2)
  b2b   [P, C]          b2[e] broadcast across partitions
  wslot [P, cap/P]      combine weight per token slot, t = mo*P + p
  xsT   [P, C/P, tb]    x^T for this core's shared-expert token block
  wsh   [P, C/P, C]     sum-over-heads shared weight Ws_eff
  bshb  [P, C]          bs_eff broadcast across partitions
Outputs:
  y     [P, cap/P, C]   wslot * (gelu(x@W1+b1) @ W2 + b2), t = mo*P + p
  sh    [P, tb/P, C]    x@Ws_eff + bs_eff for the block, t = mo*P + p
"""
import contextlib

import numpy as np

import concourse.bass as bass
import concourse.mybir as mybir
import concourse.tile as tile
from concourse import bacc
from concourse.bass_utils import run_bass_kernel_spmd

P = 128
TT = 512            # token tile (matmul free dim)
N_CORES = 8

F32 = mybir.dt.float32
F32R = mybir.dt.float32r
AF = mybir.ActivationFunctionType
ALU = mybir.AluOpType

_PROG_CACHE = {}
_LAST_IN_MAPS = None


def _build_program(C, H, cap, tb, repeat=None):
    """SPMD program for one core: expert FFN over `cap` gathered tokens plus
    shared-expert projection over a `tb`-token block. `repeat` wraps the body
    in a HW loop (benchmarking only)."""
    KO1 = C // P          # k-subtiles for matmul 1 (contraction over C)
    MO1 = H // P          # m-subtiles for matmul 1 / k-subtiles for matmul 2
    MOY = cap // P        # token m-subtiles overall
    NT = cap // TT        # token tiles (expert FFN)
    NTS = tb // TT        # token tiles (shared expert)
    MT = TT // P          # token m-subtiles per token tile

    nc = bacc.Bacc("TRN2", target_bir_lowering=False, debug=False,
                   num_devices=N_CORES)
    xT = nc.declare_dram_parameter("xT", [P, KO1, cap], F32R, isOutput=False)
    w1 = nc.declare_dram_parameter("w1", [P, KO1, H], F32R, isOutput=False)
    b1t = nc.declare_dram_parameter("b1t", [P, MO1], F32, isOutput=False)
    w2 = nc.declare_dram_parameter("w2", [P, MO1, C], F32R, isOutput=False)
    b2b = nc.declare_dram_parameter("b2b", [P, C], F32, isOutput=False)
    wslot = nc.declare_dram_parameter("wslot", [P, MOY], F32, isOutput=False)
    xsT = nc.declare_dram_parameter("xsT", [P, KO1, tb], F32R, isOutput=False)
    wsh = nc.declare_dram_parameter("wsh", [P, KO1, C], F32R, isOutput=False)
    bshb = nc.declare_dram_parameter("bshb", [P, C], F32, isOutput=False)
    y = nc.declare_dram_parameter("y", [P, MOY, C], F32, isOutput=True)
    sh = nc.declare_dram_parameter("sh", [P, tb // P, C], F32, isOutput=True)

    with tile.TileContext(nc) as tc:
        with tc.tile_pool(name="const", bufs=1) as const, \
             tc.tile_pool(name="xp", bufs=3) as xp, \
             tc.tile_pool(name="xsp", bufs=2) as xsp, \
             tc.tile_pool(name="hp", bufs=2) as hp, \
             tc.tile_pool(name="yp", bufs=4) as yp, \
             tc.tile_pool(name="psA", bufs=3, space="PSUM") as psA, \
             tc.tile_pool(name="psB", bufs=2, space="PSUM") as psB:
            # Shared-expert weights first: the shared phase runs while the
            # (much larger) expert weights stream in.
            wsh_sb = const.tile([P, KO1, C], F32R)
            nc.sync.dma_start(out=wsh_sb[:], in_=wsh[:])
            bshb_sb = const.tile([P, C], F32)
            nc.sync.dma_start(out=bshb_sb[:], in_=bshb[:])
            w1_sb = const.tile([P, KO1, H], F32R)
            nc.sync.dma_start(out=w1_sb[:], in_=w1[:])
            w2_sb = const.tile([P, MO1, C], F32R)
            nc.sync.dma_start(out=w2_sb[:], in_=w2[:])
            b1_sb = const.tile([P, MO1], F32)
            nc.sync.dma_start(out=b1_sb[:], in_=b1t[:])
            b2b_sb = const.tile([P, C], F32)
            nc.sync.dma_start(out=b2b_sb[:], in_=b2b[:])
            ws_sb = const.tile([P, MOY], F32)
            nc.sync.dma_start(out=ws_sb[:], in_=wslot[:])

            loop_cm = tc.For_i(0, repeat, 1) if repeat else contextlib.nullcontext()
            with loop_cm:
                # ---- shared expert: sh[t, :] = x_blk @ Ws_eff + bs_eff ----
                for j in range(NTS):
                    xst = xsp.tile([P, KO1, TT], F32R)
                    nc.sync.dma_start(out=xst[:],
                                      in_=xsT[:, :, j * TT:(j + 1) * TT])
                    for mt in range(MT):
                        ps = psB.tile([P, C], F32)
                        for ko in range(KO1):
                            nc.tensor.matmul(
                                ps[:],
                                lhsT=xst[:, ko, mt * P:(mt + 1) * P],
                                rhs=wsh_sb[:, ko, :],
                                start=(ko == 0), stop=(ko == KO1 - 1),
                            )
                        st = yp.tile([P, C], F32, tag="evict")
                        nc.vector.tensor_tensor(st[:], ps[:], bshb_sb[:], ALU.add)
                        nc.sync.dma_start(out=sh[:, j * MT + mt, :], in_=st[:])

                # ---- expert FFN over gathered tokens ----
                for j in range(NT):
                    xt = xp.tile([P, KO1, TT], F32R)
                    nc.sync.dma_start(out=xt[:],
                                      in_=xT[:, :, j * TT:(j + 1) * TT])
                    ht = hp.tile([P, MO1, TT], F32R)
                    for mo in range(MO1):
                        ps = psA.tile([P, TT], F32)
                        for ko in range(KO1):
                            nc.tensor.matmul(
                                ps[:],
                                lhsT=w1_sb[:, ko, mo * P:(mo + 1) * P],
                                rhs=xt[:, ko, :],
                                start=(ko == 0), stop=(ko == KO1 - 1),
                            )
                        nc.scalar.activation(
                            ht[:, mo, :], ps[:], AF.Gelu_apprx_tanh,
                            bias=b1_sb[:, mo:mo + 1], scale=1.0,
                        )
                    for mt in range(MT):
                        ps2 = psB.tile([P, C], F32)
                        for ko in range(MO1):
                            nc.tensor.matmul(
                                ps2[:],
                                lhsT=ht[:, ko, mt * P:(mt + 1) * P],
                                rhs=w2_sb[:, ko, :],
                                start=(ko == 0), stop=(ko == MO1 - 1),
                            )
                        g = j * MT + mt
                        yt = yp.tile([P, C], F32, tag="evict")
                        nc.vector.tensor_tensor(yt[:], ps2[:], b2b_sb[:], ALU.add)
                        nc.vector.tensor_scalar_mul(yt[:], yt[:],
                                                    ws_sb[:, g:g + 1])
                        nc.sync.dma_start(out=y[:, g, :], in_=yt[:])
    nc.compile()
    return nc


def _get_program(C, H, cap, tb, repeat=None):
    key = (C, H, cap, tb, repeat)
    if key not in _PROG_CACHE:
        _PROG_CACHE[key] = _build_program(C, H, cap, tb, repeat=repeat)
    return _PROG_CACHE[key]


def _ktile(a, P=128):
    """[K, M] -> [P, K/P, M] with k = ko*P + p."""
    K, M = a.shape
    return np.ascontiguousarray(a.reshape(K // P, P, M).transpose(1, 0, 2))


def kernel(x, Wg, bg, Ws, bs, W1, b1, W2, b2):
    out, _ = _run(x, Wg, bg, Ws, bs, W1, b1, W2, b2, trace=False)
    return out


def profile_once(inputs):
    """Run once with NTFF tracing; returns HW exec time in ns (or None)."""
    _, t = _run(**inputs, trace=True)
    return t


def _run(x, Wg, bg, Ws, bs, W1, b1, W2, b2, trace=False):
    global _LAST_IN_MAPS
    x = np.asarray(x, dtype=np.float32)
    Wg = np.asarray(Wg, dtype=np.float32)
    bg = np.asarray(bg, dtype=np.float32)
    Ws = np.asarray(Ws, dtype=np.float32)
    bs = np.asarray(bs, dtype=np.float32)
    W1 = np.asarray(W1, dtype=np.float32)
    b1 = np.asarray(b1, dtype=np.float32)
    W2 = np.asarray(W2, dtype=np.float32)
    b2 = np.asarray(b2, dtype=np.float32)

    B, T, C = x.shape
    E = Wg.shape[-1]
    H = W1.shape[-1]
    NS = Ws.shape[-1] // C
    KK = 2                      # top-k
    N = B * T
    TB = N // N_CORES           # shared-expert block per core
    assert E == N_CORES and C % P == 0 and H % P == 0 and TB % TT == 0

    x_flat = x.reshape(N, C)

    # ---- gate (host; part of the dispatch/sharding step) ----
    z = x_flat.astype(np.float64) @ Wg.astype(np.float64) + bg.astype(np.float64)
    s = (1.0 / (1.0 + np.exp(-z))).astype(np.float32)          # [N, E]
    idx = np.argsort(-s, axis=-1, kind='stable')[:, :KK]       # [N, K] top-k ids
    g_top = np.take_along_axis(s, idx, axis=-1)                # [N, K]
    wn = g_top / g_top.sum(axis=-1, keepdims=True)             # combine weights

    # ---- dispatch: group (token, k) pairs by expert ----
    pair_e = idx.reshape(-1)                                   # [N*K]
    order = np.argsort(pair_e, kind='stable')
    counts = np.bincount(pair_e, minlength=E)
    bounds = np.concatenate(([0], np.cumsum(counts)))
    max_cnt = int(counts.max())
    cap = max(4608, -(-max_cnt // TT) * TT)                    # capacity (mult of TT)

    nc = _get_program(C, H, cap, TB)

    Ws_eff = Ws.reshape(C, NS, C).sum(axis=1)
    bs_eff = bs.reshape(NS, C).sum(axis=0)
    wsh_t = _ktile(Ws_eff)
    bshb = np.ascontiguousarray(np.broadcast_to(bs_eff, (P, C)))
    wn_flat = wn.reshape(-1)

    in_maps = []
    sels = []
    for e in range(E):
        sel = order[bounds[e]:bounds[e + 1]]
        sels.append(sel)
        cnt = sel.shape[0]
        tok = sel // KK
        x_g = np.zeros((cap, C), dtype=np.float32)
        x_g[:cnt] = x_flat[tok]
        wv = np.zeros(cap, dtype=np.float32)
        wv[:cnt] = wn_flat[sel]
        x_blk = x_flat[e * TB:(e + 1) * TB]
        in_maps.append({
            "xT": _ktile(np.ascontiguousarray(x_g.T)),
            "w1": _ktile(W1[e]),
            "b1t": np.ascontiguousarray(b1[e].reshape(H // P, P).T),
            "w2": _ktile(W2[e]),
            "b2b": np.ascontiguousarray(np.broadcast_to(b2[e], (P, C))),
            "wslot": np.ascontiguousarray(wv.reshape(cap // P, P).T),
            "xsT": _ktile(np.ascontiguousarray(x_blk.T)),
            "wsh": wsh_t,
            "bshb": bshb,
        })

    _LAST_IN_MAPS = in_maps
    r = run_bass_kernel_spmd(nc, in_maps, core_ids=list(range(N_CORES)),
                             trace=trace)
    results = r.results

    # ---- unshard + combine ----
    y_pairs = np.empty((N * KK, C), dtype=np.float32)
    sh_all = np.empty((N, C), dtype=np.float32)
    for e in range(E):
        sel = sels[e]
        y_tok = results[e]["y"].transpose(1, 0, 2).reshape(cap, C)
        y_pairs[sel] = y_tok[:sel.shape[0]]
        sh_all[e * TB:(e + 1) * TB] = \
            results[e]["sh"].transpose(1, 0, 2).reshape(TB, C)

    route = y_pairs.reshape(N, KK, C).sum(axis=1)
    res = (x_flat + sh_all + route).reshape(B, T, C)

    # ---- load-balance stats (host, counts over N*K pairs) ----
    s_norm = s / s.sum(axis=-1, keepdims=True)
    s_top = np.take_along_axis(s_norm, idx, axis=-1)           # [N, K]
    idx_b = idx.reshape(B, T * KK)
    s_top_b = s_top.reshape(B, T * KK).astype(np.float64)
    f = np.empty((B, E), dtype=np.float32)
    p = np.empty((B, E), dtype=np.float32)
    for b in range(B):
        cnt_be = np.bincount(idx_b[b], minlength=E)
        wsum_be = np.bincount(idx_b[b], weights=s_top_b[b], minlength=E)
        f[b] = (T * KK - cnt_be).astype(np.float32)
        p[b] = (s_top_b[b].sum() - wsum_be).astype(np.float32)
    return (res, f, p), r.exec_time_ns
